# revision 44
# baseline (speedup 1.0000x reference)
"""AttentionPooling Trainium2 kernel (8 NeuronCores, Bass/Tile).

Sharding: (batch, head-group) — core c handles batch b=c//2 and heads
4*(c%2)..4*(c%2)+3. Per core, for its 4 heads:

  Q^T/K^T projections (head-dim major) run as fp8 DoubleRow matmuls
  (K=256 per matmul: HID pairs 2x128, ~2x fewer PE streaming cycles).
  Host pre-scales Wq/Wk by 16 to keep fp8e4 weights normal-range; the
  extra 256x on scores is folded into the exp scale. V stays bf16
  (V-path quantization error does not average out in the pooled output).

  Pooled attention per 128-query stripe:
    S = Q_stripe K^T (PE, bf16; 4x 512-wide matmuls into 2 PSUM tiles)
    E = exp(S/sqrt(d)/256 - 2) via ScalarE -> fp8e4 E, accum_out Z
    r = 128/Z (VectorE; 128 folded via the one-hot constant)
    w += [r0|r1]^T [E0|E1] as fp8 DoubleRow matmuls over stripe PAIRS
      (K=256 = two stripes' queries, halving w streaming cycles)

  The mean-pool is folded through the output projection, so the (B,N,HID)
  attention output is never materialized: pooled_partial =
  concat_h(w_h @ V_h) @ (Wo_slice^T/(N*128)). K-projection bias is dropped
  exactly (softmax row-shift invariance); V/output biases fold on host:
  pooled = sum(core partials) + Wo@bv + bo.

w accumulation: one PSUM bank [rows j=k-chunk, 512]; serially reused per
head (evacuated to SBUF + reopened between heads, overlapped with the
next head's stripes).

PSUM budget (8 banks): S halves 2x[128,1024]f32 (4) + w [4,512]f32 (1) +
projection chunks 3x[128,512]f32 (3).
"""

import sys

import numpy as np

for _p in ("/opt/trn_rl_repo",):
    if _p not in sys.path:
        sys.path.append(_p)

import ml_dtypes

B, N, HID = 4, 2048, 1024
HEADS, HD = 8, 128
NH = 4          # heads per core
HGW = NH * HD   # head-group width (512)
NCORES = 8
P = 128
IT = HID // P    # 8 i-tiles
PAIRS = IT // 2  # 4 DoubleRow contraction pairs
QT_TILES = N // P    # 16 query stripes
TOK_TILES = N // P   # 16 token tiles

WSCALE = 16.0    # host pre-scale on Wq/Wk (fp8 range)
RSCALE = 128.0   # scale on r = 1/Z (fp8 range), folded out of woT
SHIFT = 5.0      # exp argument shift (fp8 E range: e^(smax-SHIFT) must stay
                 # under 240 for global smax ~9.7; cancels exactly via Z)

BF16 = ml_dtypes.bfloat16
F8 = ml_dtypes.float8_e4m3

_cache = {}


def _build_nc():
    import concourse.bacc as bacc
    import concourse.tile as tile
    from concourse import mybir
    from concourse.bass import ds, ts
    from concourse.masks import make_identity

    BF = mybir.dt.bfloat16
    F32 = mybir.dt.float32
    E4 = mybir.dt.float8e4
    AF = mybir.ActivationFunctionType
    DR = mybir.MatmulPerfMode.DoubleRow

    nc = bacc.Bacc(trn_type="TRN2")

    x8_d = nc.dram_tensor("x8", (PAIRS, 2, P, N), E4, kind="ExternalInput").ap()
    wq8_d = nc.dram_tensor("wq8", (NH, PAIRS, 2, P, HD), E4, kind="ExternalInput").ap()
    wk8_d = nc.dram_tensor("wk8", (NH, PAIRS, 2, P, HD), E4, kind="ExternalInput").ap()
    xT_d = nc.dram_tensor("xT", (HID, N), BF, kind="ExternalInput").ap()
    wvT_d = nc.dram_tensor("wvT", (HID, HGW), BF, kind="ExternalInput").ap()
    woT_d = nc.dram_tensor("woT", (HGW, HID), BF, kind="ExternalInput").ap()
    bq_d = nc.dram_tensor("bq_col", (P, NH), F32, kind="ExternalInput").ap()
    out_d = nc.dram_tensor("out_pooled", (1, HID), F32, kind="ExternalOutput").ap()

    exp_scale = float(1.0 / (np.sqrt(HD) * WSCALE * WSCALE))

    with tile.TileContext(nc) as tc:
        with (
            tc.tile_pool(name="persist", bufs=1) as persist,
            tc.tile_pool(name="sp", bufs=2, space="PSUM") as sp,
            tc.tile_pool(name="wp", bufs=1, space="PSUM") as wp,
            tc.tile_pool(name="pp", bufs=3, space="PSUM") as pp,
            tc.tile_pool(name="ep", bufs=4) as ep,
            tc.tile_pool(name="rp", bufs=3) as rpool,
            tc.tile_pool(name="zp", bufs=4) as zp,
        ):
            x8_sb = persist.tile([P, PAIRS, 2, N], E4)
            wq_sb = persist.tile([P, NH, PAIRS, 2, HD], E4)
            wk_sb = persist.tile([P, NH, PAIRS, 2, HD], E4)
            x8_r = x8_d.rearrange("r k p n -> p r k n")
            wq_r = wq8_d.rearrange("h r k p d -> p h r k d")
            wk_r = wk8_d.rearrange("h r k p d -> p h r k d")
            # DMA order is the prologue critical path. The sync queue issues
            # only what the first stripes need (bias first — it gates every
            # Q evacuation); the idle GpSimd queue stuffs the rest in
            # parallel (descriptor generation is ~1-3us per DMA, serial per
            # queue).
            bq_sb = persist.tile([P, NH], F32)
            nc.sync.dma_start(out=bq_sb, in_=bq_d)
            nc.sync.dma_start(out=wk_sb[:, 0], in_=wk_r[:, 0])
            nc.sync.dma_start(out=wq_sb[:, 0], in_=wq_r[:, 0])
            for c in range(4):
                nc.sync.dma_start(
                    out=x8_sb[:, :, :, ts(c, 512)], in_=x8_r[:, :, :, ts(c, 512)]
                )
            for h in range(1, NH):
                nc.sync.dma_start(out=wq_sb[:, h], in_=wq_r[:, h])
                nc.sync.dma_start(out=wk_sb[:, h], in_=wk_r[:, h])
            # bf16 x and Wv for the V projection (needed from ~stripe 12 on)
            xT_sb = persist.tile([P, IT, N], BF)
            wv_sb = persist.tile([P, IT, HGW], BF)
            wo_sb = persist.tile([P, NH, HID], BF)
            nc.sync.dma_start(
                out=wv_sb, in_=wvT_d.rearrange("(t p) d -> p t d", p=P)
            )
            nc.sync.dma_start(
                out=xT_sb, in_=xT_d.rearrange("(t p) n -> p t n", p=P)
            )
            nc.sync.dma_start(
                out=wo_sb, in_=woT_d.rearrange("(t p) o -> p t o", p=P)
            )
            ident = persist.tile([NH, NH], BF)
            make_identity(nc, ident)
            nshift_sb = persist.tile([P, 1], F32)
            nc.vector.memset(nshift_sb, -SHIFT)
            # throwaway exp during the DMA window: loads the ScalarE
            # activation table so the first real exp skips the ~1.4us
            # ACT_TABLE_LOAD
            scratch_sb = persist.tile([P, 1], BF)
            nc.scalar.activation(
                out=scratch_sb, in_=nshift_sb, func=AF.Exp, scale=1.0
            )
            # oneh4[p, j, j'] = RSCALE iff j == j' (chunk router + r scale)
            oneh4_sb = persist.tile([P, NH, NH], BF)
            nc.vector.memset(oneh4_sb, 0.0)
            for j in range(NH):
                nc.vector.memset(oneh4_sb[:, j, j : j + 1], RSCALE)

            QT_sb = persist.tile([P, NH, N], BF)
            KT_sb = persist.tile([P, NH, N], BF)
            V_sb = persist.tile([P, TOK_TILES, HGW], BF)
            # w rows in SBUF: [chunk-row j (partition), head, 512] (k = 512j+f)
            w_sb = persist.tile([NH, NH, 512], BF)
            # per-head w^T columns: wT[h] = [128 k-part, 16 token-tiles]
            wT_sb = persist.tile([P, NH, TOK_TILES], BF)
            attT_sb = persist.tile([P, NH], BF)
            pooled_sb = persist.tile([1, HID], F32)

            def qk_chunk(proj_i, h, c, step=None, pool=None, tag="proj"):
                """One 512-token Q^T/K^T projection chunk for head h as 4
                fp8 DoubleRow matmuls (K=256 each). Generator (step=True)
                yields halfway so background work interleaves."""
                wsb, dst = ((wq_sb, QT_sb), (wk_sb, KT_sb))[proj_i]
                ps = (pool or pp).tile([P, 512], F32, tag=tag, name="ps_qk")
                for pr in range(PAIRS):
                    nc.tensor.matmul(
                        ps,
                        lhsT=wsb[:, h, pr],
                        rhs=x8_sb[:, pr, :, ts(c, 512)],
                        start=(pr == 0),
                        stop=(pr == PAIRS - 1),
                        perf_mode=DR,
                    )
                    if step and pr == 1:
                        yield
                if proj_i == 0:
                    # fused PSUM->SBUF evacuation + per-partition Q bias
                    nc.vector.tensor_tensor(
                        dst[:, h, ts(c, 512)],
                        ps,
                        bq_sb[:, h : h + 1].to_broadcast((P, 512)),
                        mybir.AluOpType.add,
                    )
                else:
                    # K bias dropped (softmax row-shift invariance)
                    nc.vector.tensor_copy(dst[:, h, ts(c, 512)], ps)
                if step:
                    yield

            def v_chunk(t, step=None):
                """One 128-token V projection tile (all 4 heads, bf16)."""
                ps = pp.tile([P, HGW], F32, tag="proj", name="ps_v")
                for i in range(IT):
                    nc.tensor.matmul(
                        ps,
                        lhsT=xT_sb[:, i, ts(t, P)],
                        rhs=wv_sb[:, i, :],
                        start=(i == 0),
                        stop=(i == IT - 1),
                    )
                    if step and i == 3:
                        yield
                nc.vector.tensor_copy(V_sb[:, t, :], ps)
                if step:
                    yield

            # ---------------- prologue: head 0's K + first Q chunk --------
            for _ in qk_chunk(1, 0, 0, pool=pp):
                pass
            for _ in qk_chunk(0, 0, 0, pool=sp, tag="s"):
                pass
            for c, (pool_, tag_) in zip(
                range(1, 4), ((pp, "proj"), (pp, "proj"), (sp, "s"))
            ):
                for _ in qk_chunk(1, 0, c, pool=pool_, tag=tag_):
                    pass

            # Background projection work, interleaved between stripes.
            bg_tasks = []
            for c in range(1, 4):
                bg_tasks.append(qk_chunk(0, 0, c, step=True))
            for h2 in range(1, NH):
                for c in range(4):
                    bg_tasks.append(qk_chunk(0, h2, c, step=True))
                    bg_tasks.append(qk_chunk(1, h2, c, step=True))
                for t in range(NH * (h2 - 1), NH * h2):
                    bg_tasks.append(v_chunk(t, step=True))
            for t in range(NH * (NH - 1), NH * NH):
                bg_tasks.append(v_chunk(t, step=True))
            bg_tasks.reverse()  # consumed LIFO-from-front via pop() below
            BG_STEPS = 2 * len(bg_tasks)  # each generator yields twice
            BG_SPREAD = 48  # finish all background work by stripe 48 of 64

            def bg_advance(si):
                lo = si * BG_STEPS // BG_SPREAD
                hi = min((si + 1) * BG_STEPS // BG_SPREAD, BG_STEPS)
                for _ in range(max(0, hi - lo)):
                    while bg_tasks:
                        try:
                            next(bg_tasks[-1])
                            break
                        except StopIteration:
                            bg_tasks.pop()

            # ---------------- pooled attention ----------------
            # w accumulator: ONE PSUM bank [4, 512] f32; k-chunk j on row j.
            # Serially reused per head (evacuate + reopen between heads).
            w_ps = wp.tile([P, 512], F32, tag="w", name="w_ps")

            def emit_S(h, qi):
                """Both k-half score matmul groups for one query stripe."""
                tiles = []
                for kk in range(2):
                    s_ps = sp.tile([P, 1024], F32, tag="s", name="s_ps")
                    for kc in range(2):
                        nc.tensor.matmul(
                            s_ps[:, ts(kc, 512)],
                            lhsT=QT_sb[:, h, ts(qi, P)],
                            rhs=KT_sb[:, h, ds(kk * 1024 + kc * 512, 512)],
                            start=True,
                            stop=True,
                        )
                    tiles.append(s_ps)
                return tiles

            def emit_w(pend):
                """One stripe-PAIR of w accumulation: 4 fp8 DoubleRow
                matmuls, K=256 = both stripes' queries."""
                e_pair, rp4, qp, _h = pend
                for j in range(4):
                    nc.tensor.matmul(
                        w_ps[0:NH, :],
                        lhsT=rp4[:, :, j, :],
                        rhs=e_pair[:, :, ts(j, 512)],
                        start=(qp == 0 and j == 0),
                        stop=(qp == QT_TILES // 2 - 1 and j == 3),
                        perf_mode=DR,
                        skip_group_check=True,
                    )

            defer_q = []

            def head_tail_evac(h):
                """After head h's last w matmul: evacuate its 4 w rows to
                SBUF (frees the bank for head h+1). The 4 transposes
                producing wT[h] are deferred one-per-stripe to keep them off
                the head-boundary critical path."""
                nc.vector.tensor_copy(w_sb[:, h, :], w_ps[0:NH, :])

                def mk(h=h, c=0):
                    pass

                for c in range(4):
                    def mk(h=h, c=c):
                        tp_ps = pp.tile([P, NH], BF, tag="proj", name="tp_ps")
                        nc.tensor.transpose(tp_ps, w_sb[:, h, ts(c, P)], ident)
                        # columns t = 4j + c of wT[h]
                        nc.vector.tensor_copy(wT_sb[:, h, c :: NH], tp_ps)
                    defer_q.append(mk)

            # Software-pipelined stripe loop (see S1 docstring): iteration
            # (h, qi) exps the previous stripe's S tiles, emits the next
            # stripe's S matmuls, then w-pair matmuls + background work.
            pend_s = emit_S(0, 0)
            pend_w = None
            e_pair = None
            rp4 = None
            for h in range(NH):
                for qi in range(QT_TILES):

                    par = qi % 2
                    if par == 0:
                        e_pair = ep.tile([P, 2, N], E4, tag="e", name="e_pair")
                        rp4 = rpool.tile([P, 2, NH, NH], E4, tag="rp", name="rp4")
                    zs = []
                    for kk, s_ps in enumerate(pend_s):
                        z_t = zp.tile([P, 1], F32, tag=f"z{kk}", name="z_t")
                        nc.scalar.activation(
                            out=e_pair[:, par, ts(kk, 1024)],
                            in_=s_ps,
                            func=AF.Exp,
                            scale=exp_scale,
                            bias=nshift_sb[:, 0:1],
                            accum_out=z_t,
                        )
                        zs.append(z_t)
                    nqi = h * QT_TILES + qi + 1
                    if nqi < NH * QT_TILES:
                        pend_s = emit_S(nqi // QT_TILES, nqi % QT_TILES)
                    r_t = zp.tile([P, 1], F32, tag="r", name="r_t")
                    nc.vector.tensor_add(r_t, zs[0], zs[1])
                    nc.vector.reciprocal(r_t, r_t)
                    # rp4[:, par, j, :] = one-hot row j scaled by RSCALE*r
                    nc.vector.tensor_tensor(
                        rp4[:, par],
                        oneh4_sb,
                        r_t.to_broadcast((P, NH, NH)),
                        mybir.AluOpType.mult,
                    )
                    if pend_w is not None and par == 1:
                        emit_w(pend_w)
                        if pend_w[2] == QT_TILES // 2 - 1:
                            head_tail_evac(pend_w[3])
                        pend_w = None
                    if par == 1:
                        pend_w = (e_pair, rp4, qi // 2, h)
                    # interleaved background projection work
                    bg_advance(h * QT_TILES + qi)
                    if defer_q:
                        defer_q.pop(0)()
            emit_w(pend_w)
            head_tail_evac(NH - 1)
            while defer_q:
                defer_q.pop(0)()

            # ---------------- tail: attended + output projection ----------
            # attT[:, h] = sum_t V[t]^T wT[h][:, t]; two head-chains run
            # interleaved so the PE pipelines their matmuls
            att_ps = [
                pp.tile([P, 1], F32, tag="proj", name="att_ps"),
                pp.tile([P, 1], F32, tag="proj", name="att_ps"),
                sp.tile([P, 1], F32, tag="s", name="att_ps2"),
                sp.tile([P, 1], F32, tag="s", name="att_ps2"),
            ]
            for t in range(TOK_TILES):
                for hh in range(NH):
                    nc.tensor.matmul(
                        att_ps[hh],
                        lhsT=V_sb[:, t, ts(hh, HD)],
                        rhs=wT_sb[:, hh, t : t + 1],
                        start=(t == 0),
                        stop=(t == TOK_TILES - 1),
                    )
            for hh in range(NH):
                nc.vector.tensor_copy(attT_sb[:, hh : hh + 1], att_ps[hh])
            p_ps = sp.tile([1, HID], F32, tag="s", name="p_ps")
            for h in range(NH):
                for oc in range(2):
                    nc.tensor.matmul(
                        p_ps[:, ts(oc, 512)],
                        lhsT=attT_sb[:, h : h + 1],
                        rhs=wo_sb[:, h, ts(oc, 512)],
                        start=(h == 0),
                        stop=(h == NH - 1),
                    )
            nc.vector.tensor_copy(pooled_sb, p_ps)
            nc.sync.dma_start(out=out_d, in_=pooled_sb)

    nc.finalize()  # Bacc: event-sem pass packs multi-waits into legal encodings
    return nc


def _get_nc():
    if "nc" not in _cache:
        _cache["nc"] = _build_nc()
    return _cache["nc"]


def _host_prep(inputs):
    """Build the 8 per-core input maps (host-side shard + transpose + cast)."""
    x = np.asarray(inputs["chunk_embeddings"], np.float32)
    in_maps = []
    for c in range(NCORES):
        b, hg = c // 2, c % 2
        sl = slice(hg * HGW, (hg + 1) * HGW)
        xT = np.ascontiguousarray(x[b].T)  # [HID, N]
        wq = np.asarray(inputs["Wq"], np.float32)[sl, :]  # [HGW, HID]
        wk = np.asarray(inputs["Wk"], np.float32)[sl, :]

        def w8(w):
            # [HGW, HID] -> lhsT pairs [NH, PAIRS, 2, P, HD], scaled
            wt = (w.T * np.float32(WSCALE)).reshape(PAIRS, 2, P, NH, HD)
            return np.ascontiguousarray(wt.transpose(3, 0, 1, 2, 4)).astype(F8)

        in_maps.append(
            {
                "x8": np.ascontiguousarray(
                    xT.reshape(PAIRS, 2, P, N)
                ).astype(F8),
                "wq8": w8(wq),
                "wk8": w8(wk),
                "xT": xT.astype(BF16),
                "wvT": np.ascontiguousarray(
                    np.asarray(inputs["Wv"], np.float32)[sl, :].T
                ).astype(BF16),
                "woT": np.ascontiguousarray(
                    np.asarray(inputs["Wo"], np.float32)[:, sl].T
                    / np.float32(N * RSCALE)
                ).astype(BF16),
                "bq_col": np.ascontiguousarray(
                    np.asarray(inputs["bq"], np.float32)[sl].reshape(NH, P).T
                    * np.float32(WSCALE)
                ),
            }
        )
    return in_maps


def _unshard(results, inputs):
    bo = np.asarray(inputs["bo"], np.float32)
    bv = np.asarray(inputs["bv"], np.float32)
    Wo = np.asarray(inputs["Wo"], np.float32)
    bv_wo = Wo @ bv  # exact fold of the V bias through the output projection
    out = np.zeros((B, HID), np.float32)
    for b in range(B):
        out[b] = (
            results[2 * b]["out_pooled"][0]
            + results[2 * b + 1]["out_pooled"][0]
            + bv_wo
            + bo
        )
    return out


def _reference_numpy(inputs):
    """Fallback for non-trivial attention masks (never hit for the spec'd
    all-ones mask): straight numpy port of the reference."""
    x = np.asarray(inputs["chunk_embeddings"], np.float32)
    mask = np.asarray(inputs["attention_mask"])
    b, n, hid = x.shape

    def proj(W, bias):
        y = x @ np.asarray(W, np.float32).T + np.asarray(bias, np.float32)
        return y.reshape(b, n, HEADS, HD).transpose(0, 2, 1, 3)

    Q = proj(inputs["Wq"], inputs["bq"])
    K = proj(inputs["Wk"], inputs["bk"])
    V = proj(inputs["Wv"], inputs["bv"])
    s = np.einsum("bhqd,bhkd->bhqk", Q, K) / np.float32(np.sqrt(HD))
    s = np.where(mask[:, None, None, :] == 0, np.float32(-1e9), s)
    s = s - s.max(axis=-1, keepdims=True)
    e = np.exp(s)
    a = e / e.sum(axis=-1, keepdims=True)
    att = np.einsum("bhqk,bhkd->bhqd", a, V)
    att = att.transpose(0, 2, 1, 3).reshape(b, n, hid)
    out = att @ np.asarray(inputs["Wo"], np.float32).T + np.asarray(
        inputs["bo"], np.float32
    )
    m = mask[:, :, None].astype(np.float32)
    return (out * m).sum(axis=1) / m.sum(axis=1)


def _run(inputs, trace=False):
    from concourse.bass_utils import run_bass_kernel_spmd

    nc = _get_nc()
    in_maps = _host_prep(inputs)
    res = run_bass_kernel_spmd(
        nc, in_maps, core_ids=list(range(NCORES)), trace=trace
    )
    _cache["last_result"] = res
    return _unshard(res.results, inputs)


def kernel(**inputs):
    mask = np.asarray(inputs["attention_mask"])
    if not np.all(mask == 1):
        return _reference_numpy(inputs)
    return _run(inputs, trace=False)


def kernel_traced(**inputs):
    """Like kernel() but with NTFF profiling; returns (out, exec_time_ns)."""
    out = _run(inputs, trace=True)
    return out, _cache["last_result"].exec_time_ns
